# revision 14
# baseline (speedup 1.0000x reference)
"""Bayesian linear layer (per-sample weights) on 8 Trainium2 NeuronCores.

out[b,o] = sum_i x[b,i] * (eps[b,i,o]*softplus(ro)[i,o] + mu[i,o])
           + eps_bias[b,o]*softplus(ro_bias)[o] + mu_bias[o]

Strategy (data-parallel over batch, 16 samples per core):
  - eps shard (16,1024,1024 f32, 64MB) is streamed in [128, 4096] tiles
    (i-chunk on partitions, 4 chunks x o on free dim; 2MB contiguous DMA),
    alternating between the two HWDGE rings (sync / scalar) so transfers
    overlap; params + small traffic ride the gpsimd (SWDGE) ring.
  - DVE multiplies each tile by the matching softplus(ro) tile, rounding
    to float32r so TensorE can consume it at full (1 cycle/row) rate.
  - TensorE reduces over i with M=1 matmuls: lhsT = x[b, chunk] column
    ([128,1] f32r), rhs = scaled eps tile slice ([128,512] f32r),
    accumulated in a [1,1024] PSUM tile per sample.
  - The x@mu term is computed once per core with M=16 fp32 matmuls and
    folded (with the bias terms) into bias16 [16,1024]; a one-hot (K=16)
    matmul adds row b of it into sample b's PSUM accumulator, and the
    scalar engine copies PSUM -> SBUF for the store.
"""

import numpy as np

import concourse.bass as bass
import concourse.bacc as bacc
import concourse.mybir as mybir
from concourse.masks import make_identity
from concourse.tile import TileContext
from concourse.bass_utils import run_bass_kernel_spmd

F32 = mybir.dt.float32
F32R = mybir.dt.float32r
AF = mybir.ActivationFunctionType

B, IN, OUT = 128, 1024, 1024
NCORES = 8
BS = B // NCORES          # 16 samples per core
P = 128                   # partitions
NCH = IN // P             # 8 i-chunks
HALF_CH = NCH // 2        # 4 chunks per eps tile
HALF_F = HALF_CH * OUT    # 4096 free elems per eps tile
NH = OUT // 512           # 2 matmul halves (PSUM bank = 512 f32)


def build_nc():
    nc = bacc.Bacc(None, target_bir_lowering=False)

    eps_d = nc.declare_dram_parameter("eps", [BS, IN, OUT], F32, isOutput=False)
    ro_d = nc.declare_dram_parameter("ro", [IN, OUT], F32, isOutput=False)
    mu_d = nc.declare_dram_parameter("mu", [IN, OUT], F32, isOutput=False)
    # xt[p, c*BS + b] = x[b, c*128 + p]  (host-side layout transform)
    xt_d = nc.declare_dram_parameter("xt", [P, NCH * BS], F32, isOutput=False)
    eb_d = nc.declare_dram_parameter("eps_bias", [BS, OUT], F32, isOutput=False)
    # ro_bias / mu_bias broadcast to BS rows on the host
    rb_d = nc.declare_dram_parameter("ro_bias16", [BS, OUT], F32, isOutput=False)
    mb_d = nc.declare_dram_parameter("mu_bias16", [BS, OUT], F32, isOutput=False)
    out_d = nc.declare_dram_parameter("out", [BS, OUT], F32, isOutput=True)

    ro_r = ro_d.rearrange("(c p) o -> p c o", p=P)
    mu_r = mu_d.rearrange("(c p) o -> p c o", p=P)

    with TileContext(nc) as tc:
        with (
            tc.tile_pool(name="const", bufs=1) as cpool,
            tc.tile_pool(name="eps", bufs=4) as epool,
            tc.tile_pool(name="epr", bufs=3) as eprpool,
            tc.tile_pool(name="small", bufs=1) as spool,
            tc.tile_pool(name="psmu", bufs=1, space="PSUM") as pmupool,
            tc.tile_pool(name="psum", bufs=3, space="PSUM") as ppool,
        ):
            # ---- softplus(ro), halves pipelined on the two HWDGE rings --
            sig = cpool.tile([P, NCH * OUT], F32)
            for h, eng in ((0, nc.sync), (1, nc.scalar)):
                sl = sig[:, h * HALF_F : (h + 1) * HALF_F]
                eng.dma_start(out=sl, in_=ro_r[:, h * HALF_CH : (h + 1) * HALF_CH, :])
                nc.scalar.activation(sl, sl, AF.Exp)
                nc.scalar.activation(sl, sl, AF.Ln, bias=1.0)

            xt = cpool.tile([P, NCH * BS], F32)
            nc.gpsimd.dma_start(out=xt, in_=xt_d[:, :])
            xtr = cpool.tile([P, NCH * BS], F32R)
            nc.vector.tensor_copy(out=xtr, in_=xt)

            ident = cpool.tile([BS, BS], F32)
            make_identity(nc, ident)
            idr = cpool.tile([BS, BS], F32R)
            nc.vector.tensor_copy(out=idr, in_=ident)

            # ---- x @ mu via M=16 matmuls (mu streamed, tiles recycled) --
            psmu = pmupool.tile([BS, OUT], F32)
            for h in range(2):
                mt = epool.tile([P, HALF_F], F32)
                nc.gpsimd.dma_start(
                    out=mt, in_=mu_r[:, h * HALF_CH : (h + 1) * HALF_CH, :]
                )
                for c4 in range(HALF_CH):
                    c = HALF_CH * h + c4
                    for nh in range(NH):
                        nc.tensor.matmul(
                            psmu[:, nh * 512 : (nh + 1) * 512],
                            xt[:, c * BS : (c + 1) * BS],
                            mt[:, c4 * OUT + nh * 512 : c4 * OUT + (nh + 1) * 512],
                            start=(c == 0),
                            stop=(c == NCH - 1),
                        )

            # ---- bias16[b,o] = eps_bias*softplus(ro_bias) + mu_bias + x@mu
            eb16 = cpool.tile([BS, OUT], F32)
            nc.gpsimd.dma_start(out=eb16, in_=eb_d[:, :])
            rb16 = cpool.tile([BS, OUT], F32)
            nc.gpsimd.dma_start(out=rb16, in_=rb_d[:, :])
            mb16 = cpool.tile([BS, OUT], F32)
            nc.gpsimd.dma_start(out=mb16, in_=mb_d[:, :])
            nc.scalar.activation(rb16, rb16, AF.Exp)
            nc.scalar.activation(rb16, rb16, AF.Ln, bias=1.0)

            nc.vector.tensor_mul(out=eb16, in0=eb16, in1=rb16)
            nc.vector.tensor_add(out=eb16, in0=eb16, in1=mb16)
            b16r = cpool.tile([BS, OUT], F32R)
            nc.vector.tensor_add(out=b16r, in0=eb16, in1=psmu)

            # ---- main streaming loop ------------------------------------
            for b in range(BS):
                ps = ppool.tile([1, OUT], F32)
                for h, eng in ((0, nc.sync), (1, nc.scalar)):
                    ep = epool.tile([P, HALF_F], F32)
                    src = eps_d[b, h * 512 : (h + 1) * 512, :]
                    eng.dma_start(
                        out=ep, in_=src.rearrange("(c p) o -> p c o", p=P)
                    )
                    for q in range(2):
                        epr = eprpool.tile([P, HALF_F // 2], F32R)
                        nc.vector.tensor_mul(
                            out=epr,
                            in0=ep[:, q * (HALF_F // 2) : (q + 1) * (HALF_F // 2)],
                            in1=sig[:, h * HALF_F + q * (HALF_F // 2) : h * HALF_F + (q + 1) * (HALF_F // 2)],
                        )
                        for c2 in range(2):
                            c = HALF_CH * h + 2 * q + c2
                            col = xtr[:, c * BS + b : c * BS + b + 1]
                            for nh in range(NH):
                                nc.tensor.matmul(
                                    ps[0:1, nh * 512 : (nh + 1) * 512],
                                    col,
                                    epr[:, c2 * OUT + nh * 512 : c2 * OUT + (nh + 1) * 512],
                                    start=(h == 0 and q == 0 and c2 == 0),
                                    stop=False,
                                )
                # one-hot matmul adds bias16[b, :] into the partition-0 PSUM row
                for nh in range(NH):
                    nc.tensor.matmul(
                        ps[0:1, nh * 512 : (nh + 1) * 512],
                        idr[:, b : b + 1],
                        b16r[:, nh * 512 : (nh + 1) * 512],
                        start=False,
                        stop=True,
                    )
                orow = spool.tile([1, OUT], F32)
                nc.scalar.copy(orow, ps[0:1, :])
                nc.gpsimd.dma_start(out=out_d[b : b + 1, :], in_=orow)

    nc.finalize()
    return nc


_NC_CACHE = None


def _get_nc():
    global _NC_CACHE
    if _NC_CACHE is None:
        _NC_CACHE = build_nc()
    return _NC_CACHE


def kernel(x, mu, ro, mu_bias, ro_bias, eps, eps_bias, _trace=False, _tmpdir=None):
    x = np.ascontiguousarray(np.asarray(x, dtype=np.float32))
    mu = np.ascontiguousarray(np.asarray(mu, dtype=np.float32))
    ro = np.ascontiguousarray(np.asarray(ro, dtype=np.float32))
    mu_bias = np.asarray(mu_bias, dtype=np.float32).reshape(1, OUT)
    ro_bias = np.asarray(ro_bias, dtype=np.float32).reshape(1, OUT)
    eps = np.asarray(eps, dtype=np.float32)
    eps_bias = np.ascontiguousarray(np.asarray(eps_bias, dtype=np.float32))

    rb16 = np.ascontiguousarray(np.broadcast_to(ro_bias, (BS, OUT)))
    mb16 = np.ascontiguousarray(np.broadcast_to(mu_bias, (BS, OUT)))

    nc = _get_nc()

    in_maps = []
    for core in range(NCORES):
        b0, b1 = core * BS, (core + 1) * BS
        x_sh = x[b0:b1]  # (BS, IN)
        # xt[p, c*BS + b] = x_sh[b, c*128 + p]
        xt = np.ascontiguousarray(
            x_sh.reshape(BS, NCH, P).transpose(2, 1, 0).reshape(P, NCH * BS)
        )
        in_maps.append(
            {
                "eps": eps[b0:b1],
                "ro": ro,
                "mu": mu,
                "xt": xt,
                "eps_bias": eps_bias[b0:b1],
                "ro_bias16": rb16,
                "mu_bias16": mb16,
            }
        )

    res = run_bass_kernel_spmd(
        nc, in_maps, core_ids=list(range(NCORES)), trace=_trace, tmpdir=_tmpdir
    )
    out = np.concatenate([res.results[c]["out"] for c in range(NCORES)], axis=0)
    if _trace:
        kernel.last_results = res
    return out


# revision 15
# speedup vs baseline: 1.2747x; 1.2747x over previous
"""Bayesian linear layer (per-sample weights) on 8 Trainium2 NeuronCores.

out[b,o] = sum_i x[b,i] * (eps[b,i,o]*softplus(ro)[i,o] + mu[i,o])
           + eps_bias[b,o]*softplus(ro_bias)[o] + mu_bias[o]

Strategy (data-parallel over batch, 16 samples per core):
  - eps shard (16,1024,1024 f32, 64MB) is streamed in [128, 4096] tiles
    (i-chunk on partitions, 4 chunks x o on free dim; 2MB contiguous DMA),
    alternating between the two HWDGE rings (sync / scalar) so transfers
    overlap; params + small traffic ride the gpsimd (SWDGE) ring.
  - DVE multiplies each tile by the matching softplus(ro) tile, rounding
    to float32r so TensorE can consume it at full (1 cycle/row) rate.
  - TensorE reduces over i with M=1 matmuls: lhsT = x[b, chunk] column
    ([128,1] f32r), rhs = scaled eps tile slice ([128,512] f32r),
    accumulated in a [1,1024] PSUM tile per sample.
  - The x@mu term is computed once per core with M=16 fp32 matmuls and
    folded (with the bias terms) into bias16 [16,1024]; a one-hot (K=16)
    matmul adds row b of it into sample b's PSUM accumulator, and the
    scalar engine copies PSUM -> SBUF for the store.
"""

import numpy as np

import concourse.bass as bass
import concourse.bacc as bacc
import concourse.mybir as mybir
from concourse.masks import make_identity
from concourse.tile import TileContext
from concourse.bass_utils import run_bass_kernel_spmd

F32 = mybir.dt.float32
F32R = mybir.dt.float32r
AF = mybir.ActivationFunctionType

B, IN, OUT = 128, 1024, 1024
NCORES = 8
BS = B // NCORES          # 16 samples per core
P = 128                   # partitions
NCH = IN // P             # 8 i-chunks
HALF_CH = NCH // 2        # 4 chunks per eps tile
HALF_F = HALF_CH * OUT    # 4096 free elems per eps tile
NH = OUT // 512           # 2 matmul halves (PSUM bank = 512 f32)


def build_nc():
    nc = bacc.Bacc(None, target_bir_lowering=False)

    eps_d = nc.declare_dram_parameter("eps", [BS, IN, OUT], F32, isOutput=False)
    ro_d = nc.declare_dram_parameter("ro", [IN, OUT], F32, isOutput=False)
    mu_d = nc.declare_dram_parameter("mu", [IN, OUT], F32, isOutput=False)
    # xt[p, c*BS + b] = x[b, c*128 + p]  (host-side layout transform)
    xt_d = nc.declare_dram_parameter("xt", [P, NCH * BS], F32, isOutput=False)
    eb_d = nc.declare_dram_parameter("eps_bias", [BS, OUT], F32, isOutput=False)
    # ro_bias / mu_bias broadcast to BS rows on the host
    rb_d = nc.declare_dram_parameter("ro_bias16", [BS, OUT], F32, isOutput=False)
    mb_d = nc.declare_dram_parameter("mu_bias16", [BS, OUT], F32, isOutput=False)
    out_d = nc.declare_dram_parameter("out", [BS, OUT], F32, isOutput=True)

    ro_r = ro_d.rearrange("(c p) o -> p c o", p=P)
    mu_r = mu_d.rearrange("(c p) o -> p c o", p=P)

    with TileContext(nc) as tc:
        with (
            tc.tile_pool(name="const", bufs=1) as cpool,
            tc.tile_pool(name="eps", bufs=4) as epool,
            tc.tile_pool(name="epr", bufs=3) as eprpool,
            tc.tile_pool(name="small", bufs=1) as spool,
            tc.tile_pool(name="psmu", bufs=1, space="PSUM") as pmupool,
            tc.tile_pool(name="psum", bufs=3, space="PSUM") as ppool,
        ):
            # ---- softplus(ro), halves pipelined on the two HWDGE rings --
            sig = cpool.tile([P, NCH * OUT], F32)
            for h, eng in ((0, nc.sync), (1, nc.scalar)):
                sl = sig[:, h * HALF_F : (h + 1) * HALF_F]
                eng.dma_start(out=sl, in_=ro_r[:, h * HALF_CH : (h + 1) * HALF_CH, :])
                nc.scalar.activation(sl, sl, AF.Exp)
                nc.scalar.activation(sl, sl, AF.Ln, bias=1.0)

            xt = cpool.tile([P, NCH * BS], F32)
            nc.gpsimd.dma_start(out=xt, in_=xt_d[:, :])
            xtr = cpool.tile([P, NCH * BS], F32R)
            nc.vector.tensor_copy(out=xtr, in_=xt)

            ident = cpool.tile([BS, BS], F32)
            make_identity(nc, ident)
            idr = cpool.tile([BS, BS], F32R)
            nc.vector.tensor_copy(out=idr, in_=ident)

            # ---- x @ mu via M=16 matmuls (mu streamed, tiles recycled) --
            psmu = pmupool.tile([BS, OUT], F32)
            for h in range(2):
                mt = epool.tile([P, HALF_F], F32)
                nc.scalar.dma_start(
                    out=mt, in_=mu_r[:, h * HALF_CH : (h + 1) * HALF_CH, :]
                )
                for c4 in range(HALF_CH):
                    c = HALF_CH * h + c4
                    for nh in range(NH):
                        nc.tensor.matmul(
                            psmu[:, nh * 512 : (nh + 1) * 512],
                            xt[:, c * BS : (c + 1) * BS],
                            mt[:, c4 * OUT + nh * 512 : c4 * OUT + (nh + 1) * 512],
                            start=(c == 0),
                            stop=(c == NCH - 1),
                        )

            # ---- bias16[b,o] = eps_bias*softplus(ro_bias) + mu_bias + x@mu
            eb16 = cpool.tile([BS, OUT], F32)
            nc.gpsimd.dma_start(out=eb16, in_=eb_d[:, :])
            rb16 = cpool.tile([BS, OUT], F32)
            nc.gpsimd.dma_start(out=rb16, in_=rb_d[:, :])
            mb16 = cpool.tile([BS, OUT], F32)
            nc.gpsimd.dma_start(out=mb16, in_=mb_d[:, :])
            nc.scalar.activation(rb16, rb16, AF.Exp)
            nc.scalar.activation(rb16, rb16, AF.Ln, bias=1.0)

            nc.vector.tensor_mul(out=eb16, in0=eb16, in1=rb16)
            nc.vector.tensor_add(out=eb16, in0=eb16, in1=mb16)
            b16r = cpool.tile([BS, OUT], F32R)
            nc.vector.tensor_add(out=b16r, in0=eb16, in1=psmu)

            # ---- main streaming loop ------------------------------------
            for b in range(BS):
                ps = ppool.tile([1, OUT], F32)
                for h in range(2):
                    ep = epool.tile([P, HALF_F], F32)
                    src = eps_d[b, h * 512 : (h + 1) * 512, :]
                    nc.sync.dma_start(
                        out=ep, in_=src.rearrange("(c p) o -> p c o", p=P)
                    )
                    for q in range(2):
                        epr = eprpool.tile([P, HALF_F // 2], F32R)
                        nc.vector.tensor_mul(
                            out=epr,
                            in0=ep[:, q * (HALF_F // 2) : (q + 1) * (HALF_F // 2)],
                            in1=sig[:, h * HALF_F + q * (HALF_F // 2) : h * HALF_F + (q + 1) * (HALF_F // 2)],
                        )
                        for c2 in range(2):
                            c = HALF_CH * h + 2 * q + c2
                            col = xtr[:, c * BS + b : c * BS + b + 1]
                            for nh in range(NH):
                                nc.tensor.matmul(
                                    ps[0:1, nh * 512 : (nh + 1) * 512],
                                    col,
                                    epr[:, c2 * OUT + nh * 512 : c2 * OUT + (nh + 1) * 512],
                                    start=(h == 0 and q == 0 and c2 == 0),
                                    stop=False,
                                )
                # one-hot matmul adds bias16[b, :] into the partition-0 PSUM row
                for nh in range(NH):
                    nc.tensor.matmul(
                        ps[0:1, nh * 512 : (nh + 1) * 512],
                        idr[:, b : b + 1],
                        b16r[:, nh * 512 : (nh + 1) * 512],
                        start=False,
                        stop=True,
                    )
                orow = spool.tile([1, OUT], F32)
                nc.scalar.copy(orow, ps[0:1, :])
                nc.gpsimd.dma_start(out=out_d[b : b + 1, :], in_=orow)

    nc.finalize()
    return nc


_NC_CACHE = None


def _get_nc():
    global _NC_CACHE
    if _NC_CACHE is None:
        _NC_CACHE = build_nc()
    return _NC_CACHE


def kernel(x, mu, ro, mu_bias, ro_bias, eps, eps_bias, _trace=False, _tmpdir=None):
    x = np.ascontiguousarray(np.asarray(x, dtype=np.float32))
    mu = np.ascontiguousarray(np.asarray(mu, dtype=np.float32))
    ro = np.ascontiguousarray(np.asarray(ro, dtype=np.float32))
    mu_bias = np.asarray(mu_bias, dtype=np.float32).reshape(1, OUT)
    ro_bias = np.asarray(ro_bias, dtype=np.float32).reshape(1, OUT)
    eps = np.asarray(eps, dtype=np.float32)
    eps_bias = np.ascontiguousarray(np.asarray(eps_bias, dtype=np.float32))

    rb16 = np.ascontiguousarray(np.broadcast_to(ro_bias, (BS, OUT)))
    mb16 = np.ascontiguousarray(np.broadcast_to(mu_bias, (BS, OUT)))

    nc = _get_nc()

    in_maps = []
    for core in range(NCORES):
        b0, b1 = core * BS, (core + 1) * BS
        x_sh = x[b0:b1]  # (BS, IN)
        # xt[p, c*BS + b] = x_sh[b, c*128 + p]
        xt = np.ascontiguousarray(
            x_sh.reshape(BS, NCH, P).transpose(2, 1, 0).reshape(P, NCH * BS)
        )
        in_maps.append(
            {
                "eps": eps[b0:b1],
                "ro": ro,
                "mu": mu,
                "xt": xt,
                "eps_bias": eps_bias[b0:b1],
                "ro_bias16": rb16,
                "mu_bias16": mb16,
            }
        )

    res = run_bass_kernel_spmd(
        nc, in_maps, core_ids=list(range(NCORES)), trace=_trace, tmpdir=_tmpdir
    )
    out = np.concatenate([res.results[c]["out"] for c in range(NCORES)], axis=0)
    if _trace:
        kernel.last_results = res
    return out
